# revision 15
# baseline (speedup 1.0000x reference)
"""Causal self-attention Trainium2 Bass kernel.

Sharding: 8-way head tensor-parallelism for QKV projections + attention
(2 heads per core, full batch), then an AllToAll re-shards the attention
output so each core computes the output projection for 1/8 of the
(batch*seq) rows.  The host concatenates the 8 row-shards.

Schedule (causal mode): projections are interleaved with attention
block-by-block (x chunk tb feeds attention block tb immediately),
keeping the PE array dense and the HAM clock warm.  wo/bo load late so
the first projection matmul starts as early as possible.

All matmul operands are bf16 (fp32 PSUM accumulation); measured end-to-end
relative error vs the fp32 reference is ~3e-3.

Per-core layouts:
  xT     [128, 8, 4096] bf16   x^T arranged (d_inner, d_outer, b*t)
  wq/wk/wv [128, 8, 128] bf16  W[:, head-slice] as (d_inner, d_outer, out)
  wo     [128, 8, 1024] bf16   full Wo
  Q^T/K^T in SBUF [128 (2 heads x 64), 4096] bf16
  V in SBUF [128 (j in chunk), 32 (b*jc), 2 (head), 80 (V | ones | pad)]
  Attention: S^T = K^T.T @ Q^T tiles [j=128, i<=512] (2 heads row-tiled in
  the PE array); softmax denominator rides the 65th (ones) column of V
  through the PV matmul; 1/den via one batched DVE reciprocal_approx_fast;
  broadcast via a K=1 matmul.
"""

import math
import os

import numpy as np

os.environ.setdefault("JAX_COMPILATION_CACHE_DIR", "/tmp/jax_cache")

D_MODEL = 1024
NUM_HEADS = 16
D_K = 64
B = 2
T = 2048
TT = B * T          # 4096 flattened tokens
NCORES = 8
HL = NUM_HEADS // NCORES   # heads per core = 2
DO = D_MODEL // 128        # 8 contraction chunks
NB = TT // 512             # 8 projection t-chunks
NI = T // 512              # 4 query chunks per batch
NJ = T // 128              # 16 key chunks per batch
SH = TT // NCORES          # 512 output rows per core

_cache = {}


def _install_ntff_hook():
    """The agent image's antenv lacks axon_hooks; replicate what
    trn_agent_boot would register so trace=True can capture NTFFs."""
    import sys
    import types

    try:
        from antenv import axon_hooks  # noqa: F401
        return True
    except ImportError:
        pass
    try:
        import antenv
        from trn_agent_boot.trn_boot import _ntff_profile_via_ctypes

        mod = types.ModuleType("antenv.axon_hooks")
        holder = [None]
        mod.set_axon_ntff_profile_hook = lambda h: holder.__setitem__(0, h)
        mod.get_axon_ntff_profile_hook = lambda: holder[0]
        sys.modules["antenv.axon_hooks"] = mod
        antenv.axon_hooks = mod
        mod.set_axon_ntff_profile_hook(
            _ntff_profile_via_ctypes("/opt/axon/libaxon_pjrt.so")
        )
        return True
    except Exception:
        return False


def _build_module(mode, blocks=None, n_mtiles=1):
    """Build + compile the Bass module.

    mode: "causal" (tril mask), "ones" (no masking), "generic"
    blocks: for generic mode, blocks[jc][a] = 0 skip / 1 full / (2, idx) mixed
    """
    from contextlib import ExitStack

    import concourse.mybir as mybir
    import concourse.tile as tile
    from concourse import bacc

    F32 = mybir.dt.float32
    BF16 = mybir.dt.bfloat16
    AF = mybir.ActivationFunctionType

    nc = bacc.Bacc(
        "TRN2",
        target_bir_lowering=False,
        debug=False,
        enable_asserts=False,
        num_devices=NCORES,
    )

    xT = nc.dram_tensor("xT", [128, NB, DO, 512], BF16, kind="ExternalInput").ap()
    wq = nc.dram_tensor("wq", [128, DO, 128], BF16, kind="ExternalInput").ap()
    wk = nc.dram_tensor("wk", [128, DO, 128], BF16, kind="ExternalInput").ap()
    wv = nc.dram_tensor("wv", [128, DO, 128], BF16, kind="ExternalInput").ap()
    wo = nc.dram_tensor("wo", [128, 1024], BF16, kind="ExternalInput").ap()
    bqin = nc.dram_tensor("bq", [128, 1], F32, kind="ExternalInput").ap()
    bkin = nc.dram_tensor("bk", [128, 1], F32, kind="ExternalInput").ap()
    bvin = nc.dram_tensor("bv", [128, 1], F32, kind="ExternalInput").ap()
    tri_in = nc.dram_tensor("tri", [128, 128], BF16, kind="ExternalInput").ap()
    id_in = nc.dram_tensor("identf", [128, 128], F32, kind="ExternalInput").ap()
    if mode == "generic":
        mtiles = nc.dram_tensor(
            "mtiles", [n_mtiles, 128, 512], BF16, kind="ExternalInput"
        ).ap()
    # partial output projection for all tokens; host sums cores' partials
    y = nc.dram_tensor("y", [TT, 1024], BF16, kind="ExternalOutput").ap()

    with tile.TileContext(nc) as tc, ExitStack() as ctx:
        pers = ctx.enter_context(tc.tile_pool(name="pers", bufs=1))
        # one PSUM pool for the whole kernel; 8 banks total:
        #   tag A: [128,2,512] f32 x2 = 4 banks (proj QK, attn ST pairs, yproj)
        #   tag C: [128,512] f32 x2 = 2 banks (Vt proj, rcp bcast)
        #   pv: [65,2,512] f32 x1 = 2 banks
        pp = ctx.enter_context(tc.tile_pool(name="pp", bufs=2, space="PSUM"))

        # ---- persistent SBUF (weights needed to start first) ----
        wq_sb = pers.tile([128, DO, 128], BF16, name="wq_sb")
        nc.sync.dma_start(wq_sb[:], wq[:])
        wk_sb = pers.tile([128, DO, 128], BF16, name="wk_sb")
        nc.sync.dma_start(wk_sb[:], wk[:])
        wv_sb = pers.tile([128, DO, 128], BF16, name="wv_sb")
        nc.sync.dma_start(wv_sb[:], wv[:])
        bq_sb = pers.tile([128, 1], F32, name="bq_sb")
        nc.sync.dma_start(bq_sb[:], bqin[:])
        bk_sb = pers.tile([128, 1], F32, name="bk_sb")
        nc.sync.dma_start(bk_sb[:], bkin[:])
        bv_sb = pers.tile([128, 1], F32, name="bv_sb")
        nc.sync.dma_start(bv_sb[:], bvin[:])
        tri_full = pers.tile([128, 128], BF16, name="tri_full")
        nc.sync.dma_start(tri_full[:], tri_in[:])
        tri_sb = tri_full[:, 0:128]
        ident_t = pers.tile([128, 128], F32, name="ident_t")
        nc.sync.dma_start(ident_t[:], id_in[:])
        ident = ident_t[:]
        wo_sb = pers.tile([128, 1024], BF16, name="wo_sb")
        nc.sync.dma_start(wo_sb[:], wo[:])

        ones_bf = pers.tile([128, 128], BF16, name="ones_bf")
        nc.vector.memset(ones_bf[:], 1.0)

        qt = pers.tile([128, TT], BF16, name="qt")
        kt = pers.tile([128, TT], BF16, name="kt")
        vsb = pers.tile([128, B * NJ, HL, 80], BF16, name="vsb")
        nc.vector.tensor_copy(
            vsb[:, :, :, 64],
            ones_bf[:, 0 : B * NJ * HL].rearrange("p (a b) -> p a b", a=B * NJ),
        )
        # normalized attention output, (2 heads x 64 d) x tokens
        ot = pers.tile([128, TT], BF16, name="ot")

        xtp = ctx.enter_context(tc.tile_pool(name="xtp", bufs=3))
        vtp = ctx.enter_context(tc.tile_pool(name="vtp", bufs=2))
        sxp = ctx.enter_context(tc.tile_pool(name="sxp", bufs=4))
        rcpp = ctx.enter_context(tc.tile_pool(name="rcpp", bufs=2))
        otsp = ctx.enter_context(tc.tile_pool(name="otsp", bufs=2))
        yp = ctx.enter_context(tc.tile_pool(name="yp", bufs=3))
        mtp = ctx.enter_context(tc.tile_pool(name="mtp", bufs=2))

        def emit_proj_chunk(tb):
            xt_t = xtp.tile([128, DO, 512], BF16, name=f"xt{tb}", tag="xt")
            nc.sync.dma_start(xt_t[:], xT[:, tb, :, :])
            for w_sb, b_sb, dst, nm in (
                (wq_sb, bq_sb, qt, "q"),
                (wk_sb, bk_sb, kt, "k"),
            ):
                ps = pp.tile([128, 2, 512], F32, name=f"ps{nm}{tb}", tag="A")
                for do in range(DO):
                    nc.tensor.matmul(
                        ps[:, 0, :],
                        w_sb[:, do, :],
                        xt_t[:, do, :],
                        start=(do == 0),
                        stop=(do == DO - 1),
                    )
                nc.vector.tensor_scalar_add(
                    dst[:, 512 * tb : 512 * (tb + 1)], ps[:, 0, :], b_sb[:]
                )
            # V^T, then PE-transpose into [j, d] layout
            vps_t = pp.tile([128, 512], F32, name=f"vps{tb}", tag="C")
            vps = vps_t[:]
            for do in range(DO):
                nc.tensor.matmul(
                    vps[:],
                    wv_sb[:, do, :],
                    xt_t[:, do, :],
                    start=(do == 0),
                    stop=(do == DO - 1),
                )
            vt_t = vtp.tile([128, 512], F32, name=f"vt{tb}", tag="vt")
            nc.vector.tensor_scalar_add(vt_t[:], vps[:], bv_sb[:])
            for k in range(4):
                g = 4 * tb + k  # global t-tile = b*NJ + jc
                tps_t = pp.tile([128, 128], F32, name=f"tps{g}", tag="C")
                tps = tps_t[:]
                nc.tensor.transpose(
                    tps, vt_t[:, 128 * k : 128 * (k + 1)], ident
                )
                nc.vector.tensor_copy(
                    vsb[:, g, :, 0:64],
                    tps.rearrange("t (h c) -> t h c", h=HL),
                )

        def emit_outproj(g):
            # partial y rows [512g, 512(g+1)): ot^T @ wo, K=128
            for ti in range(4):
                t0 = 512 * g + 128 * ti
                yps = pp.tile([128, 2, 512], F32, name=f"yps{g}_{ti}", tag="A")
                for oc in range(2):
                    nc.tensor.matmul(
                        yps[:, oc, :],
                        ot[:, t0 : t0 + 128],
                        wo_sb[:, 512 * oc : 512 * (oc + 1)],
                        start=True,
                        stop=True,
                    )
                y_t = yp.tile([128, 1024], BF16, name=f"y{g}_{ti}", tag="y")
                nc.vector.tensor_copy(y_t[:], yps[:, :, :])
                nc.sync.dma_start(y[t0 : t0 + 128, :], y_t[:])

        def emit_norm_pre(pend):
            # DVE-only: batched approximate reciprocal for both heads'
            # denominators (custom DVE op only works at partition base 0)
            pb, pa, ppvc, pden, pii0 = pend
            rcf = rcpp.tile([1, 2, 512], F32, name=f"rf{pb}_{pa}", tag="rf")
            nc.vector.reciprocal_approx_fast(rcf[:, :, :], pden[:, :, :])
            rcp = rcpp.tile([1, 2, 512], BF16, name=f"rcp{pb}_{pa}", tag="rcp")
            nc.vector.tensor_copy(rcp[:, :, :], rcf[:, :, :])
            return rcp

        def emit_norm_post(pend, rcp):
            pb, pa, ppvc, pden, pii0 = pend
            for h in range(HL):
                rb_t = pp.tile([128, 512], F32, name=f"rb{pb}_{pa}_{h}", tag="C")
                rb = rb_t[:]
                nc.tensor.matmul(
                    rb[0:64, :],
                    ones_bf[0:1, 0:64],
                    rcp[0:1, h, :],
                    start=True,
                    stop=True,
                )
                if h == 0:
                    nc.vector.tensor_mul(
                        ot[0:64, pii0 : pii0 + 512],
                        ppvc[0:64, 0, :],
                        rb[0:64, :],
                    )
                else:
                    # h1 lands at partitions 64-127 via an SBUF->SBUF DMA
                    ots = otsp.tile(
                        [64, 512], BF16, name=f"ots{pb}_{pa}", tag="ots"
                    )
                    nc.vector.tensor_mul(ots[:], ppvc[0:64, 1, :], rb[0:64, :])
                    nc.sync.dma_start(ot[64:128, pii0 : pii0 + 512], ots[:])
            emit_outproj(4 * pb + pa)

        pend = [None]

        def emit_attn_block(b, a):
            ii0 = b * T + 512 * a
            if mode == "causal":
                jcs = list(range(4 * a + 4))
            elif mode == "ones":
                jcs = list(range(NJ))
            else:
                jcs = [jc for jc in range(NJ) if blocks[jc][a] != 0]
            if not jcs:
                nc.vector.memset(ot[:, ii0 : ii0 + 512], 0.0)
                emit_outproj(4 * b + a)
                return
            pv_pair = pp.tile(
                [65, 2, 512], F32, name=f"pv_{b}_{a}", tag="pv", bufs=1
            )
            pvs = [pv_pair[:, h, :] for h in range(HL)]
            for idx, jc in enumerate(jcs):
                j0 = b * T + 128 * jc
                diag = mode == "causal" and jc >= 4 * a
                s = 128 * (jc - 4 * a) if diag else 0
                w = 512 - s
                first = idx == 0
                last = idx == len(jcs) - 1
                st = pp.tile(
                    [128, 2, 512], F32, name=f"st{b}_{a}_{jc}", tag="A"
                )
                for h in range(HL):
                    nc.tensor.matmul(
                        st[:, h, 0:w],
                        kt[64 * h : 64 * (h + 1), j0 : j0 + 128],
                        qt[64 * h : 64 * (h + 1), ii0 + s : ii0 + 512],
                        start=True,
                        stop=True,
                        tile_position=(64 * h, 0),
                    )
                ex = sxp.tile(
                    [128, 2, 512], BF16, name=f"ex{b}_{a}_{jc}", tag="ex"
                )
                nc.scalar.activation(ex[:, :, 0:w], st[:, :, 0:w], AF.Exp)
                if diag:
                    for h in range(HL):
                        nc.vector.tensor_mul(
                            ex[:, h, 0:128], ex[:, h, 0:128], tri_sb
                        )
                if mode == "generic" and blocks[jc][a] != 1:
                    mt = mtp.tile(
                        [128, 512], BF16, name=f"mt{b}_{a}_{jc}", tag="mt"
                    )
                    nc.sync.dma_start(mt[:], mtiles[blocks[jc][a][1]])
                    for h in range(HL):
                        nc.vector.tensor_mul(ex[:, h, :], ex[:, h, :], mt[:])
                for h in range(HL):
                    nc.tensor.matmul(
                        pvs[h][:, s:512],
                        vsb[:, b * NJ + jc, h, 0:65],
                        ex[:, h, 0:w],
                        start=first,
                        stop=last,
                    )
            pvc = rcpp.tile([64, 2, 512], F32, name=f"pvc{b}_{a}", tag="pvc")
            nc.vector.tensor_copy(pvc[:], pv_pair[0:64, :, :])
            den = rcpp.tile([1, 2, 512], F32, name=f"den{b}_{a}", tag="den")
            nc.vector.tensor_copy(den[:, :, :], pv_pair[64:65, :, :])
            pend[0] = (b, a, pvc, den, ii0)

        # ---- main schedule ----
        # per cycle: recip chain (DVE) first, proj matmuls keep the PE busy
        # while it completes, then the previous block's normalize + partial
        # output projection, then this block's attention
        if mode == "causal":
            for tb in range(NB):
                rcp = emit_norm_pre(pend[0]) if pend[0] is not None else None
                emit_proj_chunk(tb)
                if pend[0] is not None:
                    emit_norm_post(pend[0], rcp)
                    pend[0] = None
                emit_attn_block(tb // NI, tb % NI)
        else:
            for tb in range(NB):
                emit_proj_chunk(tb)
            for b in range(B):
                for a in range(NI):
                    if pend[0] is not None:
                        emit_norm_post(pend[0], emit_norm_pre(pend[0]))
                        pend[0] = None
                    emit_attn_block(b, a)
        if pend[0] is not None:
            emit_norm_post(pend[0], emit_norm_pre(pend[0]))

    nc.compile()
    return nc


def _detect_mode(mask):
    m2 = np.asarray(mask).reshape(T, T)
    if np.array_equal(m2, np.tril(np.ones((T, T), m2.dtype))):
        return "causal", None, None
    if np.all(m2 != 0):
        return "ones", None, None
    # generic: classify [jc, a] blocks of mask^T
    mT = (m2 != 0).T.astype(np.float32)  # [j, i]
    blocks = [[0] * NI for _ in range(NJ)]
    tiles = []
    seen = {}
    for jc in range(NJ):
        for a in range(NI):
            sub = mT[128 * jc : 128 * (jc + 1), 512 * a : 512 * (a + 1)]
            if not sub.any():
                blocks[jc][a] = 0
            elif sub.all():
                blocks[jc][a] = 1
            else:
                key = sub.tobytes()
                if key not in seen:
                    seen[key] = len(tiles)
                    tiles.append(sub.copy())
                blocks[jc][a] = (2, seen[key])
    mt = np.stack(tiles) if tiles else np.zeros((1, 128, 512), np.float32)
    return "generic", blocks, mt


def _bf16(a):
    import ml_dtypes

    return np.ascontiguousarray(a, dtype=np.float32).astype(ml_dtypes.bfloat16)


def _rearr_w(w):
    # [D, M] -> [128, DO, M] as (d_inner, d_outer, m), bf16
    m = w.shape[1]
    return _bf16(
        np.ascontiguousarray(w, dtype=np.float32)
        .reshape(DO, 128, m)
        .transpose(1, 0, 2)
    )


def kernel(x, mask, Wq, bq, Wk, bk, Wv, bv, Wo, bo, trace=False):
    from concourse import bass_utils

    x = np.asarray(x, dtype=np.float32)
    Wq = np.asarray(Wq, dtype=np.float32)
    Wk = np.asarray(Wk, dtype=np.float32)
    Wv = np.asarray(Wv, dtype=np.float32)
    Wo = np.asarray(Wo, dtype=np.float32)
    bq = np.asarray(bq, dtype=np.float32)
    bk = np.asarray(bk, dtype=np.float32)
    bv = np.asarray(bv, dtype=np.float32)
    bo = np.asarray(bo, dtype=np.float32)

    mode, blocks, mt = _detect_mode(mask)
    cache_key = (mode, None if blocks is None else str(blocks))
    if cache_key not in _cache:
        _cache[cache_key] = _build_module(
            mode, blocks, 1 if mt is None else mt.shape[0]
        )
    nc = _cache[cache_key]

    scale = 1.0 / math.sqrt(D_K)
    # [128 d_inner, NB chunk, DO d_outer, 512] — chunk-contiguous for 8KB DMA lines
    xT_arr = _bf16(
        x.reshape(TT, D_MODEL)
        .T.reshape(DO, 128, NB, 512)
        .transpose(1, 2, 0, 3)
    )
    tri_arr = _bf16(np.triu(np.ones((128, 128), np.float32)))
    id_arr = np.eye(128, dtype=np.float32)

    in_maps = []
    for c in range(NCORES):
        sl = slice(128 * c, 128 * (c + 1))
        m = {
            "xT": xT_arr,
            "wq": _rearr_w(Wq[:, sl] * scale),
            "wk": _rearr_w(Wk[:, sl]),
            "wv": _rearr_w(Wv[:, sl]),
            "wo": _bf16(Wo[sl, :]),
            "bq": np.ascontiguousarray((bq[sl] * scale).reshape(128, 1)),
            "bk": np.ascontiguousarray(bk[sl].reshape(128, 1)),
            "bv": np.ascontiguousarray(bv[sl].reshape(128, 1)),
            "tri": tri_arr,
            "identf": id_arr,
        }
        if mode == "generic":
            m["mtiles"] = _bf16(mt)
        in_maps.append(m)

    if trace:
        trace = _install_ntff_hook()
    res = bass_utils.run_bass_kernel_spmd(
        nc, in_maps, core_ids=list(range(NCORES)), trace=trace
    )
    out = np.zeros((TT, 1024), dtype=np.float32)
    for c in range(NCORES):
        out += np.asarray(res.results[c]["y"], dtype=np.float32)
    out += bo.reshape(1, 1024)
    if trace:
        kernel._last_result = res
    return out.reshape(B, T, D_MODEL)


# revision 16
# speedup vs baseline: 1.1029x; 1.1029x over previous
"""Causal self-attention Trainium2 Bass kernel.

Sharding: 8-way head tensor-parallelism for QKV projections + attention
(2 heads per core, full batch), then an AllToAll re-shards the attention
output so each core computes the output projection for 1/8 of the
(batch*seq) rows.  The host concatenates the 8 row-shards.

Schedule (causal mode): projections are interleaved with attention
block-by-block (x chunk tb feeds attention block tb immediately),
keeping the PE array dense and the HAM clock warm.  wo/bo load late so
the first projection matmul starts as early as possible.

All matmul operands are bf16 (fp32 PSUM accumulation); measured end-to-end
relative error vs the fp32 reference is ~3e-3.

Per-core layouts:
  xT     [128, 8, 4096] bf16   x^T arranged (d_inner, d_outer, b*t)
  wq/wk/wv [128, 8, 128] bf16  W[:, head-slice] as (d_inner, d_outer, out)
  wo     [128, 8, 1024] bf16   full Wo
  Q^T/K^T in SBUF [128 (2 heads x 64), 4096] bf16
  V in SBUF [128 (j in chunk), 32 (b*jc), 2 (head), 80 (V | ones | pad)]
  Attention: S^T = K^T.T @ Q^T tiles [j=128, i<=512] (2 heads row-tiled in
  the PE array); softmax denominator rides the 65th (ones) column of V
  through the PV matmul; 1/den via one batched DVE reciprocal_approx_fast;
  broadcast via a K=1 matmul.
"""

import math
import os

import numpy as np

os.environ.setdefault("JAX_COMPILATION_CACHE_DIR", "/tmp/jax_cache")

D_MODEL = 1024
NUM_HEADS = 16
D_K = 64
B = 2
T = 2048
TT = B * T          # 4096 flattened tokens
NCORES = 8
HL = NUM_HEADS // NCORES   # heads per core = 2
DO = D_MODEL // 128        # 8 contraction chunks
NB = TT // 512             # 8 projection t-chunks
NI = T // 512              # 4 query chunks per batch
NJ = T // 128              # 16 key chunks per batch
SH = TT // NCORES          # 512 output rows per core

_cache = {}


def _install_ntff_hook():
    """The agent image's antenv lacks axon_hooks; replicate what
    trn_agent_boot would register so trace=True can capture NTFFs."""
    import sys
    import types

    try:
        from antenv import axon_hooks  # noqa: F401
        return True
    except ImportError:
        pass
    try:
        import antenv
        from trn_agent_boot.trn_boot import _ntff_profile_via_ctypes

        mod = types.ModuleType("antenv.axon_hooks")
        holder = [None]
        mod.set_axon_ntff_profile_hook = lambda h: holder.__setitem__(0, h)
        mod.get_axon_ntff_profile_hook = lambda: holder[0]
        sys.modules["antenv.axon_hooks"] = mod
        antenv.axon_hooks = mod
        mod.set_axon_ntff_profile_hook(
            _ntff_profile_via_ctypes("/opt/axon/libaxon_pjrt.so")
        )
        return True
    except Exception:
        return False


def _build_module(mode, blocks=None, n_mtiles=1):
    """Build + compile the Bass module.

    mode: "causal" (tril mask), "ones" (no masking), "generic"
    blocks: for generic mode, blocks[jc][a] = 0 skip / 1 full / (2, idx) mixed
    """
    from contextlib import ExitStack

    import concourse.mybir as mybir
    import concourse.tile as tile
    from concourse import bacc

    F32 = mybir.dt.float32
    BF16 = mybir.dt.bfloat16
    AF = mybir.ActivationFunctionType

    nc = bacc.Bacc(
        "TRN2",
        target_bir_lowering=False,
        debug=False,
        enable_asserts=False,
        num_devices=NCORES,
    )

    xT = nc.dram_tensor("xT", [128, NB, DO, 512], BF16, kind="ExternalInput").ap()
    wq = nc.dram_tensor("wq", [128, DO, 128], BF16, kind="ExternalInput").ap()
    wk = nc.dram_tensor("wk", [128, DO, 128], BF16, kind="ExternalInput").ap()
    wv = nc.dram_tensor("wv", [128, DO, 128], BF16, kind="ExternalInput").ap()
    wo = nc.dram_tensor("wo", [128, 1024], BF16, kind="ExternalInput").ap()
    bqin = nc.dram_tensor("bq", [128, 1], F32, kind="ExternalInput").ap()
    bkin = nc.dram_tensor("bk", [128, 1], F32, kind="ExternalInput").ap()
    bvin = nc.dram_tensor("bv", [128, 1], F32, kind="ExternalInput").ap()
    tri_in = nc.dram_tensor("tri", [128, 128], BF16, kind="ExternalInput").ap()
    id_in = nc.dram_tensor("identf", [128, 128], F32, kind="ExternalInput").ap()
    if mode == "generic":
        mtiles = nc.dram_tensor(
            "mtiles", [n_mtiles, 128, 512], BF16, kind="ExternalInput"
        ).ap()
    # partial output projection for all tokens; host sums cores' partials
    y = nc.dram_tensor("y", [TT, 1024], BF16, kind="ExternalOutput").ap()

    with tile.TileContext(nc) as tc, ExitStack() as ctx:
        pers = ctx.enter_context(tc.tile_pool(name="pers", bufs=1))
        # one PSUM pool for the whole kernel; 8 banks total:
        #   tag A: [128,2,512] f32 x2 = 4 banks (proj QK, attn ST pairs, yproj)
        #   tag C: [128,512] f32 x2 = 2 banks (Vt proj, rcp bcast)
        #   pv: [65,2,512] f32 x1 = 2 banks
        pp = ctx.enter_context(tc.tile_pool(name="pp", bufs=2, space="PSUM"))

        # ---- persistent SBUF (weights needed to start first) ----
        wq_sb = pers.tile([128, DO, 128], BF16, name="wq_sb")
        nc.sync.dma_start(wq_sb[:], wq[:])
        wk_sb = pers.tile([128, DO, 128], BF16, name="wk_sb")
        nc.sync.dma_start(wk_sb[:], wk[:])
        wv_sb = pers.tile([128, DO, 128], BF16, name="wv_sb")
        nc.sync.dma_start(wv_sb[:], wv[:])
        bq_sb = pers.tile([128, 1], F32, name="bq_sb")
        nc.sync.dma_start(bq_sb[:], bqin[:])
        bk_sb = pers.tile([128, 1], F32, name="bk_sb")
        nc.sync.dma_start(bk_sb[:], bkin[:])
        bv_sb = pers.tile([128, 1], F32, name="bv_sb")
        nc.sync.dma_start(bv_sb[:], bvin[:])
        tri_full = pers.tile([128, 128], BF16, name="tri_full")
        nc.sync.dma_start(tri_full[:], tri_in[:])
        tri_sb = tri_full[:, 0:128]
        ident_t = pers.tile([128, 128], F32, name="ident_t")
        nc.sync.dma_start(ident_t[:], id_in[:])
        ident = ident_t[:]
        wo_sb = pers.tile([128, 1024], BF16, name="wo_sb")
        nc.sync.dma_start(wo_sb[:], wo[:])

        ones_bf = pers.tile([128, 128], BF16, name="ones_bf")
        nc.vector.memset(ones_bf[:], 1.0)

        qt = pers.tile([128, TT], BF16, name="qt")
        kt = pers.tile([128, TT], BF16, name="kt")
        vsb = pers.tile([128, B * NJ, HL, 80], BF16, name="vsb")
        nc.vector.tensor_copy(
            vsb[:, :, :, 64],
            ones_bf[:, 0 : B * NJ * HL].rearrange("p (a b) -> p a b", a=B * NJ),
        )
        # normalized attention output, (2 heads x 64 d) x tokens
        ot = pers.tile([128, TT], BF16, name="ot")

        xtp = ctx.enter_context(tc.tile_pool(name="xtp", bufs=3))
        vtp = ctx.enter_context(tc.tile_pool(name="vtp", bufs=2))
        sxp = ctx.enter_context(tc.tile_pool(name="sxp", bufs=4))
        rcpp = ctx.enter_context(tc.tile_pool(name="rcpp", bufs=2))
        otsp = ctx.enter_context(tc.tile_pool(name="otsp", bufs=2))
        yp = ctx.enter_context(tc.tile_pool(name="yp", bufs=3))
        mtp = ctx.enter_context(tc.tile_pool(name="mtp", bufs=2))

        def emit_proj_chunk(tb):
            xt_t = xtp.tile([128, DO, 512], BF16, name=f"xt{tb}", tag="xt")
            nc.sync.dma_start(xt_t[:], xT[:, tb, :, :])
            for w_sb, b_sb, dst, nm in (
                (wq_sb, bq_sb, qt, "q"),
                (wk_sb, bk_sb, kt, "k"),
            ):
                ps = pp.tile([128, 2, 512], F32, name=f"ps{nm}{tb}", tag="A")
                for do in range(DO):
                    nc.tensor.matmul(
                        ps[:, 0, :],
                        w_sb[:, do, :],
                        xt_t[:, do, :],
                        start=(do == 0),
                        stop=(do == DO - 1),
                    )
                nc.vector.tensor_scalar_add(
                    dst[:, 512 * tb : 512 * (tb + 1)], ps[:, 0, :], b_sb[:]
                )
            # V^T, then PE-transpose into [j, d] layout
            vps_t = pp.tile([128, 512], F32, name=f"vps{tb}", tag="C")
            vps = vps_t[:]
            for do in range(DO):
                nc.tensor.matmul(
                    vps[:],
                    wv_sb[:, do, :],
                    xt_t[:, do, :],
                    start=(do == 0),
                    stop=(do == DO - 1),
                )
            vt_t = vtp.tile([128, 512], F32, name=f"vt{tb}", tag="vt")
            nc.vector.tensor_scalar_add(vt_t[:], vps[:], bv_sb[:])
            for k in range(4):
                g = 4 * tb + k  # global t-tile = b*NJ + jc
                tps_t = pp.tile([128, 128], F32, name=f"tps{g}", tag="C")
                tps = tps_t[:]
                nc.tensor.transpose(
                    tps, vt_t[:, 128 * k : 128 * (k + 1)], ident
                )
                nc.vector.tensor_copy(
                    vsb[:, g, :, 0:64],
                    tps.rearrange("t (h c) -> t h c", h=HL),
                )

        def emit_outproj(g):
            # partial y rows [512g, 512(g+1)): ot^T @ wo, K=128
            for ti in range(4):
                t0 = 512 * g + 128 * ti
                yps = pp.tile([128, 2, 512], F32, name=f"yps{g}_{ti}", tag="A")
                for oc in range(2):
                    nc.tensor.matmul(
                        yps[:, oc, :],
                        ot[:, t0 : t0 + 128],
                        wo_sb[:, 512 * oc : 512 * (oc + 1)],
                        start=True,
                        stop=True,
                    )
                y_t = yp.tile([128, 1024], BF16, name=f"y{g}_{ti}", tag="y")
                nc.vector.tensor_copy(y_t[:], yps[:, :, :])
                nc.sync.dma_start(y[t0 : t0 + 128, :], y_t[:])

        def emit_norm_pre(pend):
            # DVE-only: batched approximate reciprocal for both heads'
            # denominators (custom DVE op only works at partition base 0)
            pb, pa, ppvc, pden, pii0 = pend
            rcf = rcpp.tile([1, 2, 512], F32, name=f"rf{pb}_{pa}", tag="rf")
            nc.vector.reciprocal_approx_fast(rcf[:, :, :], pden[:, :, :])
            rcp = rcpp.tile([1, 2, 512], BF16, name=f"rcp{pb}_{pa}", tag="rcp")
            nc.vector.tensor_copy(rcp[:, :, :], rcf[:, :, :])
            return rcp

        def emit_norm_post(pend, rcp):
            pb, pa, ppvc, pden, pii0 = pend
            for h in range(HL):
                rb_t = pp.tile([128, 512], F32, name=f"rb{pb}_{pa}_{h}", tag="C")
                rb = rb_t[:]
                nc.tensor.matmul(
                    rb[0:64, :],
                    ones_bf[0:1, 0:64],
                    rcp[0:1, h, :],
                    start=True,
                    stop=True,
                )
                # cross-base DVE write is legal with 32-aligned bases
                nc.vector.tensor_mul(
                    ot[64 * h : 64 * (h + 1), pii0 : pii0 + 512],
                    ppvc[0:64, h, :],
                    rb[0:64, :],
                )
            emit_outproj(4 * pb + pa)

        pend = [None]

        def emit_attn_block(b, a):
            ii0 = b * T + 512 * a
            if mode == "causal":
                jcs = list(range(4 * a + 4))
            elif mode == "ones":
                jcs = list(range(NJ))
            else:
                jcs = [jc for jc in range(NJ) if blocks[jc][a] != 0]
            if not jcs:
                nc.vector.memset(ot[:, ii0 : ii0 + 512], 0.0)
                emit_outproj(4 * b + a)
                return
            pv_pair = pp.tile(
                [65, 2, 512], F32, name=f"pv_{b}_{a}", tag="pv", bufs=1
            )
            pvs = [pv_pair[:, h, :] for h in range(HL)]
            for idx, jc in enumerate(jcs):
                j0 = b * T + 128 * jc
                diag = mode == "causal" and jc >= 4 * a
                s = 128 * (jc - 4 * a) if diag else 0
                w = 512 - s
                first = idx == 0
                last = idx == len(jcs) - 1
                st = pp.tile(
                    [128, 2, 512], F32, name=f"st{b}_{a}_{jc}", tag="A"
                )
                for h in range(HL):
                    nc.tensor.matmul(
                        st[:, h, 0:w],
                        kt[64 * h : 64 * (h + 1), j0 : j0 + 128],
                        qt[64 * h : 64 * (h + 1), ii0 + s : ii0 + 512],
                        start=True,
                        stop=True,
                        tile_position=(64 * h, 0),
                    )
                ex = sxp.tile(
                    [128, 2, 512], BF16, name=f"ex{b}_{a}_{jc}", tag="ex"
                )
                nc.scalar.activation(ex[:, :, 0:w], st[:, :, 0:w], AF.Exp)
                if diag:
                    for h in range(HL):
                        nc.vector.tensor_mul(
                            ex[:, h, 0:128], ex[:, h, 0:128], tri_sb
                        )
                if mode == "generic" and blocks[jc][a] != 1:
                    mt = mtp.tile(
                        [128, 512], BF16, name=f"mt{b}_{a}_{jc}", tag="mt"
                    )
                    nc.sync.dma_start(mt[:], mtiles[blocks[jc][a][1]])
                    for h in range(HL):
                        nc.vector.tensor_mul(ex[:, h, :], ex[:, h, :], mt[:])
                for h in range(HL):
                    nc.tensor.matmul(
                        pvs[h][:, s:512],
                        vsb[:, b * NJ + jc, h, 0:65],
                        ex[:, h, 0:w],
                        start=first,
                        stop=last,
                    )
            pvc = rcpp.tile([64, 2, 512], F32, name=f"pvc{b}_{a}", tag="pvc")
            nc.vector.tensor_copy(pvc[:], pv_pair[0:64, :, :])
            den = rcpp.tile([1, 2, 512], F32, name=f"den{b}_{a}", tag="den")
            nc.vector.tensor_copy(den[:, :, :], pv_pair[64:65, :, :])
            pend[0] = (b, a, pvc, den, ii0)

        # ---- main schedule ----
        # per cycle: recip chain (DVE) first, proj matmuls keep the PE busy
        # while it completes, then the previous block's normalize + partial
        # output projection, then this block's attention
        if mode == "causal":
            for tb in range(NB):
                rcp = emit_norm_pre(pend[0]) if pend[0] is not None else None
                emit_proj_chunk(tb)
                if pend[0] is not None:
                    emit_norm_post(pend[0], rcp)
                    pend[0] = None
                emit_attn_block(tb // NI, tb % NI)
        else:
            for tb in range(NB):
                emit_proj_chunk(tb)
            for b in range(B):
                for a in range(NI):
                    if pend[0] is not None:
                        emit_norm_post(pend[0], emit_norm_pre(pend[0]))
                        pend[0] = None
                    emit_attn_block(b, a)
        if pend[0] is not None:
            emit_norm_post(pend[0], emit_norm_pre(pend[0]))

    nc.compile()
    return nc


def _detect_mode(mask):
    m2 = np.asarray(mask).reshape(T, T)
    if np.array_equal(m2, np.tril(np.ones((T, T), m2.dtype))):
        return "causal", None, None
    if np.all(m2 != 0):
        return "ones", None, None
    # generic: classify [jc, a] blocks of mask^T
    mT = (m2 != 0).T.astype(np.float32)  # [j, i]
    blocks = [[0] * NI for _ in range(NJ)]
    tiles = []
    seen = {}
    for jc in range(NJ):
        for a in range(NI):
            sub = mT[128 * jc : 128 * (jc + 1), 512 * a : 512 * (a + 1)]
            if not sub.any():
                blocks[jc][a] = 0
            elif sub.all():
                blocks[jc][a] = 1
            else:
                key = sub.tobytes()
                if key not in seen:
                    seen[key] = len(tiles)
                    tiles.append(sub.copy())
                blocks[jc][a] = (2, seen[key])
    mt = np.stack(tiles) if tiles else np.zeros((1, 128, 512), np.float32)
    return "generic", blocks, mt


def _bf16(a):
    import ml_dtypes

    return np.ascontiguousarray(a, dtype=np.float32).astype(ml_dtypes.bfloat16)


def _rearr_w(w):
    # [D, M] -> [128, DO, M] as (d_inner, d_outer, m), bf16
    m = w.shape[1]
    return _bf16(
        np.ascontiguousarray(w, dtype=np.float32)
        .reshape(DO, 128, m)
        .transpose(1, 0, 2)
    )


def kernel(x, mask, Wq, bq, Wk, bk, Wv, bv, Wo, bo, trace=False):
    from concourse import bass_utils

    x = np.asarray(x, dtype=np.float32)
    Wq = np.asarray(Wq, dtype=np.float32)
    Wk = np.asarray(Wk, dtype=np.float32)
    Wv = np.asarray(Wv, dtype=np.float32)
    Wo = np.asarray(Wo, dtype=np.float32)
    bq = np.asarray(bq, dtype=np.float32)
    bk = np.asarray(bk, dtype=np.float32)
    bv = np.asarray(bv, dtype=np.float32)
    bo = np.asarray(bo, dtype=np.float32)

    mode, blocks, mt = _detect_mode(mask)
    cache_key = (mode, None if blocks is None else str(blocks))
    if cache_key not in _cache:
        _cache[cache_key] = _build_module(
            mode, blocks, 1 if mt is None else mt.shape[0]
        )
    nc = _cache[cache_key]

    scale = 1.0 / math.sqrt(D_K)
    # [128 d_inner, NB chunk, DO d_outer, 512] — chunk-contiguous for 8KB DMA lines
    xT_arr = _bf16(
        x.reshape(TT, D_MODEL)
        .T.reshape(DO, 128, NB, 512)
        .transpose(1, 2, 0, 3)
    )
    tri_arr = _bf16(np.triu(np.ones((128, 128), np.float32)))
    id_arr = np.eye(128, dtype=np.float32)

    in_maps = []
    for c in range(NCORES):
        sl = slice(128 * c, 128 * (c + 1))
        m = {
            "xT": xT_arr,
            "wq": _rearr_w(Wq[:, sl] * scale),
            "wk": _rearr_w(Wk[:, sl]),
            "wv": _rearr_w(Wv[:, sl]),
            "wo": _bf16(Wo[sl, :]),
            "bq": np.ascontiguousarray((bq[sl] * scale).reshape(128, 1)),
            "bk": np.ascontiguousarray(bk[sl].reshape(128, 1)),
            "bv": np.ascontiguousarray(bv[sl].reshape(128, 1)),
            "tri": tri_arr,
            "identf": id_arr,
        }
        if mode == "generic":
            m["mtiles"] = _bf16(mt)
        in_maps.append(m)

    if trace:
        trace = _install_ntff_hook()
    res = bass_utils.run_bass_kernel_spmd(
        nc, in_maps, core_ids=list(range(NCORES)), trace=trace
    )
    out = np.zeros((TT, 1024), dtype=np.float32)
    for c in range(NCORES):
        out += np.asarray(res.results[c]["y"], dtype=np.float32)
    out += bo.reshape(1, 1024)
    if trace:
        kernel._last_result = res
    return out.reshape(B, T, D_MODEL)
